# revision 65
# baseline (speedup 1.0000x reference)
"""AffinityLoss Trainium2 kernel.

loss = mean_b( ||x_b x_b^T||_F^2 + ||y_b y_b^T||_F^2 - 2 ||x_b y_b^T||_F^2 )

with x_b (20, N), y_b (4, N), N = 257*400 = 102800.

Strategy: stack z = [x; y] (24, N) per batch.  With sign vector
sigma = (+1)*20 ++ (-1)*4 and G = z z^T (24, 24):

    loss_b = sum_{d,e} sigma_d sigma_e G[d,e]^2

Data-parallel over batch: 2 batches per core on 8 cores.

The tensor engine contracts over the partition axis, so the Gram contraction
needs n on partitions.  Instead of transposing on-chip (a second full pass
through the PE), the host pre-folds z into

    zF[b, p, c, r] = z[b, r, 128*c + p]      (p: partition, c: chunk, r: row)

(and casts fp32->bf16, zero-padding n to a whole number of chunks).  Each
(128, 5*24) slice of a DMA'd tile is then directly a stack of five
partition-major n-chunks, and one syrk matmul per 5 chunks accumulates all
their 24x24 Gram contributions into a (128, 120) PSUM tile (5 diagonal
24x24 blocks are real; off-diagonal blocks and rows 120..127 — fed by the
128-column widened stationary operand that keeps fast-weight-load eligible —
are ignored cross terms).  Loads alternate between the SP and ACT HWDGE
rings so per-DMA descriptor-generation overheads overlap.  Each core writes
out its two 120x120 accumulators; the host sums the diagonal blocks and
does the tiny signed square-sum + mean.

bf16 inputs halve HBM traffic and double PE streaming; the loss stays
within ~1e-5 relative of the fp32 reference.
"""

import os
import sys

import numpy as np

_TRN_REPO = "/opt/trn_rl_repo"
if os.path.isdir(_TRN_REPO) and _TRN_REPO not in sys.path:
    sys.path.insert(0, _TRN_REPO)

B, D, S, H, W = 16, 20, 4, 257, 400
N = H * W                  # 102800
R = D + S                  # 24 z-rows
NCORES = 8
BPC = B // NCORES          # 2 batches per core
KPACK = 5                  # n-chunks per matmul (5*24 = 120 <= 128 cols)
PPART = KPACK * R          # 120
CHUNKS = 805               # ceil(102800/128)=804, padded to a multiple of 5
NPAD = CHUNKS * 128        # 103040
# per-batch DMA tiling (in chunks); each tile's chunk count is a multiple of
# KPACK so matmuls never span tiles (each list sums to CHUNKS=805).
# batch 0 leads with a small tile (fast pipeline fill); batch 1 ends with a
# small tile (short epilogue after the last DMA lands).
TILE_CHUNKS_B = (
    [25] + [60] * 13,
    [60] * 13 + [25],
)
MAXT = 60
FIRST_SPLIT = 1            # optional extra split of the very first tile

_nc_cache = None


def _build():
    global _nc_cache
    if _nc_cache is not None:
        return _nc_cache

    import concourse.mybir as mybir
    import concourse.tile as tile
    from concourse import bacc

    f32 = mybir.dt.float32
    bf16 = mybir.dt.bfloat16
    nc = bacc.Bacc("TRN2", target_bir_lowering=False)
    z_t = nc.dram_tensor("z", (BPC, 128, CHUNKS * R), bf16, kind="ExternalInput")
    out_t = nc.dram_tensor("out", (BPC, PPART, PPART), f32, kind="ExternalOutput")

    with tile.TileContext(nc) as tc:
        with (
            tc.tile_pool(name="zf_pool", bufs=10) as zf_pool,
            tc.tile_pool(name="misc_pool", bufs=2) as misc_pool,
            tc.tile_pool(name="pg_pool", bufs=2, space="PSUM") as pg_pool,
        ):
            for b in range(BPC):
                zb = z_t[b]
                # 128 partitions: rows 120..127 catch the harmless extra
                # output rows of the widened-stationary matmuls (see below).
                g_acc = pg_pool.tile([128, PPART], f32, name=f"gacc{b}", tag="gacc")
                tiles = TILE_CHUNKS_B[b]
                first = True
                c0 = 0
                for t, tch in enumerate(tiles):
                    tf = tch * R
                    zf = zf_pool.tile([128, tf], bf16, name="zf", tag="zf",
                                      padded_shape=[128, MAXT * R])
                    src = zb[:, c0 * R:(c0 + tch) * R]
                    if b == 0 and t == 0 and FIRST_SPLIT > 1:
                        # split the pipeline-filling first load
                        QF = tf // FIRST_SPLIT
                        for qq in range(FIRST_SPLIT):
                            f1 = (qq + 1) * QF if qq < FIRST_SPLIT - 1 else tf
                            nc.sync.dma_start(
                                zf[:, qq * QF:f1], src[:, qq * QF:f1]
                            )
                    else:
                        # alternate the two HWDGE rings (SP / ACT): the
                        # per-DMA sequencer+DGE overheads run in parallel
                        eng = nc.sync if t % 2 == 0 else nc.scalar
                        eng.dma_start(zf[:, :], src)
                    n_mm = tch // KPACK
                    # in the final tile, run groups in reverse so the LAST
                    # matmul is group 0 — always wide — and its stop closes
                    # the full 128-partition accumulation region
                    final_tile = t == len(tiles) - 1
                    order = range(n_mm - 1, -1, -1) if final_tile else range(n_mm)
                    for idx, m in enumerate(order):
                        f0 = m * PPART
                        # widen the stationary operand to 128 columns when the
                        # tile has 8 spare columns after this group: a full
                        # 128-column weight enables the PE's fast-weight-load
                        # so LDWEIGHTS never bounds the matmul stream.  The 8
                        # extra output rows (120..127) land in g_acc rows the
                        # host never reads.
                        wide = f0 + 128 <= tf
                        lw = 128 if wide else PPART
                        sl_w = zf[:, f0:f0 + lw]
                        sl_m = zf[:, f0:f0 + PPART]
                        last = final_tile and (idx == n_mm - 1)
                        nc.tensor.matmul(g_acc[0:lw, :], sl_w, sl_m,
                                         start=first, stop=last)
                        first = False
                    c0 += tch

                # evacuate the Gram accumulator; host does the tiny reduce
                gsb = misc_pool.tile([PPART, PPART], f32, name="gsb", tag="gsb")
                nc.vector.tensor_copy(gsb[:], g_acc[0:PPART, :])
                nc.sync.dma_start(out_t[b], gsb[:])

    nc.finalize()
    _nc_cache = nc
    return nc


def _make_in_maps(input, target):
    import ml_dtypes

    input = np.asarray(input, dtype=np.float32).reshape(B, D, N)
    target = np.asarray(target, dtype=np.float32).reshape(B, S, N)
    z = np.concatenate([input, target], axis=1).astype(ml_dtypes.bfloat16)
    zp = np.zeros((B, R, NPAD), dtype=ml_dtypes.bfloat16)
    zp[:, :, :N] = z
    # (B, R, CHUNKS, 128) -> (B, 128, CHUNKS, R): each 128-chunk becomes
    # partition-major with rows on the free axis.
    zf = np.ascontiguousarray(zp.reshape(B, R, CHUNKS, 128).transpose(0, 3, 2, 1))
    zf = zf.reshape(B, 128, CHUNKS * R)
    in_maps = []
    for c in range(NCORES):
        in_maps.append({"z": np.ascontiguousarray(zf[c * BPC:(c + 1) * BPC])})
    return in_maps


def _host_reduce(results):
    total = np.float64(0.0)
    for r in results:
        gout = np.asarray(r["out"], dtype=np.float64)  # (BPC, 120, 120)
        for b in range(BPC):
            blocks = gout[b].reshape(KPACK, R, KPACK, R)
            G = sum(blocks[i, :, i, :] for i in range(KPACK))  # (24, 24)
            total += np.sum(G * G) - 4.0 * np.sum(G[:D, D:] ** 2)
    total /= B
    return np.asarray(total, dtype=np.float32).reshape(())


def run(input, target, trace=False, **kwargs):
    """Run the SPMD kernel on cores 0..7; returns (scalar_loss, BassKernelResults)."""
    import time

    from concourse.bass_utils import run_bass_kernel_spmd

    nc = _build()
    in_maps = _make_in_maps(input, target)
    try:
        res = run_bass_kernel_spmd(
            nc, in_maps, core_ids=list(range(NCORES)), trace=trace, **kwargs
        )
    except Exception:
        # transient accelerator states (e.g. a prior crashed process) have
        # been observed to clear after ~30s; retry once
        time.sleep(30)
        res = run_bass_kernel_spmd(
            nc, in_maps, core_ids=list(range(NCORES)), trace=trace, **kwargs
        )
    return _host_reduce(res.results), res


def kernel(input, target):
    loss, _ = run(input, target, trace=False)
    return loss


if __name__ == "__main__":
    rng = np.random.default_rng(0)
    inp = rng.standard_normal((B, D, H, W), dtype=np.float32)
    tgt = rng.standard_normal((B, S, H, W), dtype=np.float32)
    got = kernel(input=inp, target=tgt)
    x = inp.reshape(B, D, -1).astype(np.float64)
    y = tgt.reshape(B, S, -1).astype(np.float64)
    gxx = np.einsum("bdn,ben->bde", x, x)
    gyy = np.einsum("bsn,btn->bst", y, y)
    gxy = np.einsum("bdn,bsn->bds", x, y)
    want = np.mean(
        (gxx ** 2).sum((1, 2)) + (gyy ** 2).sum((1, 2)) - 2 * (gxy ** 2).sum((1, 2))
    )
    print("got", got, "want", want, "rel", abs(got - want) / abs(want))
